# revision 1
# baseline (speedup 1.0000x reference)
"""KimiMoEGate (sigmoid scoring, group-limited top-k) on 8 Trainium2 cores.

Strategy (hardcoded for hidden_states [4,4096,2048], weight [256,2048]):
  - Token-parallel: 16384 tokens sharded 2048/core across 8 cores; router
    weight + bias replicated per core.
  - Host prep per core: x^T tiles (h on partitions) split into fp16 hi/lo;
    W^T scaled by 1024 split into fp16 hi/lo. Router logits computed on PE as
    3 fp16 passes (xh*wh + xh*wl + xl*wh) accumulated in fp32 PSUM ->
    ~fp32-accurate logits at 1 cycle/row instead of fp32's 4.
  - ACT applies sigmoid (scale 2^-10 folds away the 1024). DVE does the
    group-limited top-k with the native max8 / max_index / match_replace ops.
  - Per-expert weights recovered with scalar_tensor_tensor:
    w_j = sum_e (masked[e] == m8[j]) * scores[e], then normalized on-chip.
"""

import numpy as np

from concourse import bacc, bass_utils
import concourse.mybir as mybir
from concourse.tile import TileContext

F16 = mybir.dt.float16
F32 = mybir.dt.float32
U16 = mybir.dt.uint16
I32 = mybir.dt.int32
AF = mybir.ActivationFunctionType
ALU = mybir.AluOpType
AX = mybir.AxisListType

N_CORES = 8
N_GROUP = 8
EXP_PER_GROUP = 32
E = 256
H = 2048
H_CHUNKS = 16  # 2048 / 128
T_TOTAL = 16384
T_CORE = T_TOTAL // N_CORES
N_TILES = T_CORE // 128  # 16


def build_kernel(nc, n_tiles=N_TILES):
    xh = nc.dram_tensor("xh", [n_tiles, 128, H_CHUNKS, 128], F16, kind="ExternalInput").ap()
    xl = nc.dram_tensor("xl", [n_tiles, 128, H_CHUNKS, 128], F16, kind="ExternalInput").ap()
    wh = nc.dram_tensor("wh", [128, H_CHUNKS, E], F16, kind="ExternalInput").ap()
    wl = nc.dram_tensor("wl", [128, H_CHUNKS, E], F16, kind="ExternalInput").ap()
    bias = nc.dram_tensor("bias_rep", [128, E], F32, kind="ExternalInput").ap()
    idx_out = nc.dram_tensor("idx_out", [n_tiles, 128, 8], I32, kind="ExternalOutput").ap()
    wt_out = nc.dram_tensor("wt_out", [n_tiles, 128, 8], F32, kind="ExternalOutput").ap()

    with TileContext(nc) as tc:
        with (
            tc.tile_pool(name="const", bufs=1) as cpool,
            tc.tile_pool(name="xin", bufs=3) as xpool,
            tc.tile_pool(name="work", bufs=2) as wpool,
            tc.tile_pool(name="psum", bufs=2, space="PSUM") as ppool,
            tc.tile_pool(name="persist", bufs=1) as perspool,
        ):
            wh_sb = cpool.tile([128, H_CHUNKS, E], F16)
            wl_sb = cpool.tile([128, H_CHUNKS, E], F16)
            bias_in = cpool.tile([128, E], F32)
            bias_sb = cpool.tile([128, E], F32)
            nc.sync.dma_start(wh_sb, wh)
            nc.sync.dma_start(wl_sb, wl)
            nc.sync.dma_start(bias_in, bias)
            # DVE copy so in-loop DVE consumers depend on a DVE producer
            # (program order) instead of carrying a DMA-sem wait.
            nc.vector.tensor_copy(bias_sb, bias_in)

            idx_i32 = perspool.tile([128, n_tiles, 8], I32)
            w_raw = perspool.tile([128, n_tiles, 8], F32)

            for i in range(n_tiles):
                xh_sb = xpool.tile([128, H_CHUNKS, 128], F16, tag="xh")
                xl_sb = xpool.tile([128, H_CHUNKS, 128], F16, tag="xl")
                nc.sync.dma_start(xh_sb, xh[i])
                nc.sync.dma_start(xl_sb, xl[i])

                ps = ppool.tile([128, E], F32)
                for ho in range(H_CHUNKS):
                    nc.tensor.matmul(ps, xh_sb[:, ho, :], wh_sb[:, ho, :],
                                     start=(ho == 0), stop=False)
                    nc.tensor.matmul(ps, xh_sb[:, ho, :], wl_sb[:, ho, :],
                                     start=False, stop=False)
                    nc.tensor.matmul(ps, xl_sb[:, ho, :], wh_sb[:, ho, :],
                                     start=False, stop=(ho == H_CHUNKS - 1))

                # scores = sigmoid(logits); psum holds 1024*logits
                scores = wpool.tile([128, E], F32, tag="scores")
                nc.scalar.activation(scores, ps, AF.Sigmoid, scale=float(2.0 ** -10))

                # scores_for_choice = scores + bias
                sb = wpool.tile([128, E], F32, tag="sb")
                nc.vector.tensor_add(sb, scores, bias_sb)
                sbg = sb.rearrange("p (g e) -> p g e", g=N_GROUP)

                # top-2 per group of 32 -> group scores
                g1 = wpool.tile([128, N_GROUP], F32, tag="g1")
                nc.vector.reduce_max(g1, sbg, axis=AX.X)
                kn = wpool.tile([128, E], F32, tag="kn")
                nc.vector.match_replace(out=kn, in_to_replace=g1, in_values=sb,
                                        imm_value=-1e30)
                g2 = wpool.tile([128, N_GROUP], F32, tag="g2")
                nc.vector.reduce_max(g2, kn.rearrange("p (g e) -> p g e", g=N_GROUP),
                                     axis=AX.X)
                gs = wpool.tile([128, N_GROUP], F32, tag="gs")
                nc.vector.tensor_add(gs, g1, g2)

                # top-4 groups: threshold at 4th largest of the 8 group scores
                g8 = wpool.tile([128, 8], F32, tag="g8")
                nc.vector.max(out=g8, in_=gs)
                gm = wpool.tile([128, N_GROUP], F32, tag="gm")
                nc.vector.tensor_scalar(gm, gs, g8[:, 3:4], None, op0=ALU.is_ge)

                # mask the biased scores and take top-8
                tmp = wpool.tile([128, N_GROUP, EXP_PER_GROUP], F32, tag="tmp")
                nc.vector.tensor_mul(tmp, sbg,
                                     gm.unsqueeze(2).to_broadcast([128, N_GROUP, EXP_PER_GROUP]))
                tmpf = tmp.rearrange("p g e -> p (g e)")
                m8 = wpool.tile([128, 8], F32, tag="m8")
                nc.vector.max(out=m8, in_=tmpf)
                i8 = wpool.tile([128, 8], U16, tag="i8")
                nc.vector.max_index(i8, m8, tmpf)
                nc.vector.tensor_copy(idx_i32[:, i, :], i8)

                # gather unbiased scores at the top-8 positions:
                # w_j = sum_e (tmp[e] == m8[j]) * scores[e]
                junk = wpool.tile([128, E], F32, tag="junk")
                for j in range(8):
                    nc.vector.scalar_tensor_tensor(
                        out=junk, in0=tmpf, scalar=m8[:, j:j + 1], in1=scores,
                        op0=ALU.is_equal, op1=ALU.mult,
                        accum_out=w_raw[:, i, j:j + 1])

            # normalize: w / (sum + 1e-20) * 2.5
            denom = perspool.tile([128, n_tiles], F32)
            nc.vector.reduce_sum(denom, w_raw, axis=AX.X)
            nc.vector.tensor_scalar_add(denom, denom, 1e-20)
            recip = perspool.tile([128, n_tiles], F32)
            nc.vector.reciprocal(recip, denom)
            wnorm = perspool.tile([128, n_tiles, 8], F32)
            nc.vector.tensor_mul(wnorm, w_raw,
                                 recip.unsqueeze(2).to_broadcast([128, n_tiles, 8]))
            nc.vector.tensor_scalar_mul(wnorm, wnorm, 2.5)

            nc.sync.dma_start(idx_out.rearrange("t p k -> p t k"), idx_i32)
            nc.sync.dma_start(wt_out.rearrange("t p k -> p t k"), wnorm)
    return nc


def prep_core_inputs(x_core, wh_, wl_, bias_rep):
    n_tiles = x_core.shape[0] // 128
    x = np.ascontiguousarray(x_core, dtype=np.float32)
    xh = x.astype(np.float16)
    xl = (x - xh.astype(np.float32)).astype(np.float16)

    def tile_x(a):
        # [T, H] -> [n_tiles, 128p(h_inner), 16(h_outer), 128(t)]
        return np.ascontiguousarray(
            a.reshape(n_tiles, 128, H_CHUNKS, 128).transpose(0, 3, 2, 1))

    return {"xh": tile_x(xh), "xl": tile_x(xl),
            "wh": wh_, "wl": wl_, "bias_rep": bias_rep}


def prep_shared(weight, bias_vec):
    ws = np.ascontiguousarray(weight, dtype=np.float32) * 1024.0
    wh_ = ws.astype(np.float16)
    wl_ = (ws - wh_.astype(np.float32)).astype(np.float16)

    def tile_w(a):
        # [E, H] -> [H, E] -> [128p(h_inner), 16(h_outer), E]
        return np.ascontiguousarray(a.T.reshape(H_CHUNKS, 128, E).transpose(1, 0, 2))

    bias_rep = np.broadcast_to(np.asarray(bias_vec, np.float32), (128, E)).copy()
    return tile_w(wh_), tile_w(wl_), bias_rep


_CACHED = {}


def _get_nc():
    if "nc" not in _CACHED:
        nc = bacc.Bacc("TRN2", num_devices=N_CORES)
        build_kernel(nc)
        nc.compile()
        _CACHED["nc"] = nc
    return _CACHED["nc"]


def make_in_maps(hidden_states, weight, e_score_correction_bias):
    x = np.asarray(hidden_states, np.float32).reshape(-1, H)
    wh_, wl_, bias_rep = prep_shared(np.asarray(weight, np.float32),
                                     np.asarray(e_score_correction_bias, np.float32))
    return [prep_core_inputs(x[c * T_CORE:(c + 1) * T_CORE], wh_, wl_, bias_rep)
            for c in range(N_CORES)]


def kernel(hidden_states, weight, e_score_correction_bias):
    in_maps = make_in_maps(hidden_states, weight, e_score_correction_bias)
    nc = _get_nc()
    res = bass_utils.run_bass_kernel_spmd(nc, in_maps, core_ids=list(range(N_CORES)))
    idx = np.concatenate([r["idx_out"].reshape(-1, 8) for r in res.results], axis=0)
    wt = np.concatenate([r["wt_out"].reshape(-1, 8) for r in res.results], axis=0)
    return idx.astype(np.int32), wt.astype(np.float32)
